# revision 22
# baseline (speedup 1.0000x reference)
"""Trainium2 Bass kernel for nn_Attention_730144440595 (NormAttention block).

8 NeuronCores, data-parallel over batch (16 -> 2/core). Per core:
  - channel-LN folded: x centered on-device (mean via PE ones-matmuls, broadcast
    via K=1 matmul); per-pixel rstd folded into the softmax exp as a log-bias.
  - QKV GEMM produces q (2 o-tiles) and zero-padded per-head k (4 o-tiles);
    v is produced TRANSPOSED (v^T[j, h*64+d]) by a second small GEMM so the
    attn@V contraction needs no DMA transposes.
  - q L2-norm applied via PE-broadcast multiply; k-norm and the x8 scale fold
    into the softmax exp's per-partition scale, per-pixel rstd into its bias.
  - attention transposed (sim^T[j,i]); exp emitted directly in fp8-e5m2,
    most on ACT (native Exp) and some on DVE (Schraudolph bit-trick:
    u8 = trunc(logit*4*log2(e) + 60.5) reinterpreted as e5m2).
  - attn@V in fp8 DoubleRow mode (K=256 per instruction over jc pairs) with
    e4m3 v^T augmented by a std column so the softmax denominator drops out
    of the same matmul.  Out-proj in bf16 (K=64/head), f32 PSUM everywhere.
"""

import sys
import types

import numpy as np

B = 2
C = 256
N = 1024
HEADS = 4
D = 64
P = 128
NCORES = 8
LN_EPS = 1e-5
LOG8 = float(np.log(8.0))
LOG2E4 = float(4.0 / np.log(2.0))          # 5.770780163555851
LOG8L = float(np.log(8.0 * 4.0 / np.log(2.0)))  # log(8 * 4*log2(e))
NQKV = 6         # qkv o-tiles: q 0-1, k(padded) 2-5
OW = 1024        # weight cols: q 0-255, k-pad 256-767, vT 768-1023
# which jc's exp runs on DVE (bit-trick) instead of ACT, per head
DVE_JCS = ((1,), (4,), (2,), (6,))


def _host_consts():
    cst = np.zeros((P, 16), np.float32)
    # E_ind[cc][p, h] = 1 iff h == 2*cc + p//64   (cols 0-3 / 4-7)
    for cc in range(2):
        for p in range(P):
            cst[p, 4 * cc + 2 * cc + p // 64] = 1.0
    cst[:, 8] = 1.0 / 256.0    # rhs_x col0 (mean)
    cst[:, 11] = 1.0 / 256.0   # rhs_q col1 (msq)  (cols 10-11)
    cst[:, 12] = -1.0 / 256.0  # negones
    cst[:64, 13] = 1.0         # khalf indicator (rows 0-63)
    cst[:, 14] = 1.0           # ones col
    cst[64:, 15] = 1.0         # khalf_odd (rows 64-127)
    cst4 = np.zeros((HEADS, 2 * P), np.float32)
    for cc in range(2):
        for m in range(P):
            cst4[2 * cc + m // 64, cc * P + m] = 1.0
    return cst, cst4


def _host_weights(w_qkv, w_out, g):
    """wt_pad [C, 1024]: q cols 0-255, k head h in its own zero-padded 128-col
    block (256+128h..), vT cols 768-1023 = w_v^T laid out (c, h*64+d).
    g folded in on host.  wot_pad [512, C]: head h rows 128h..128h+64."""
    import ml_dtypes
    wg = np.asarray(w_qkv, np.float32) * np.asarray(g, np.float32).reshape(1, C)
    wt = np.ascontiguousarray(wg.T)  # [C, 768]
    wt_pad = np.zeros((C, OW), np.float32)
    wt_pad[:, 0:256] = wt[:, 0:256]
    for h in range(HEADS):
        off = 256 + 128 * h + (h % 2) * D
        wt_pad[:, off:off + D] = wt[:, 256 + D * h:256 + D * (h + 1)]
    wt_pad[:, 768:1024] = wt[:, 512:768]
    wot = np.ascontiguousarray(np.asarray(w_out, np.float32).T)  # [ci, co]
    wot_pad = np.zeros((4 * P, C), np.float32)
    for h in range(HEADS):
        wot_pad[128 * h:128 * h + D, :] = wot[D * h:D * (h + 1), :]
    return wt_pad.astype(ml_dtypes.bfloat16), wot_pad.astype(ml_dtypes.bfloat16)


def _install_ntff_hook():
    try:
        import antenv
        if getattr(antenv, "axon_hooks", None) is not None:
            return
        from trn_agent_boot.trn_boot import _ntff_profile_via_ctypes
        hook = _ntff_profile_via_ctypes('/opt/axon/libaxon_pjrt.so')
        mod = types.ModuleType('antenv.axon_hooks')
        mod._hook = hook
        mod.get_axon_ntff_profile_hook = lambda: mod._hook
        mod.set_axon_ntff_profile_hook = lambda h: setattr(mod, '_hook', h)
        sys.modules['antenv.axon_hooks'] = mod
        antenv.axon_hooks = mod
    except Exception:
        pass


def build_nc(num_devices=NCORES):
    import concourse.bass as bass
    import concourse.tile as tile
    import concourse.mybir as mybir
    from concourse import bacc
    from contextlib import ExitStack

    dt = mybir.dt
    f32 = dt.float32
    bf16 = dt.bfloat16
    f8e4 = dt.float8e4
    f8e5 = dt.float8e5
    u8 = dt.uint8
    AF = mybir.ActivationFunctionType
    OP = mybir.AluOpType
    DR = mybir.MatmulPerfMode.DoubleRow

    # Steer the ACT table selector: keep Exp/Ln only in the combined set so
    # the kernel never thrashes between exp_and_others and natural_log sets
    # (each switch costs ~2.6us of ACT_TABLE_LOAD + drain).
    from concourse.hw_specs import get_activation_tables
    _tabs = get_activation_tables("gen3")
    for _name, _fns in _tabs.items():
        if _name != "natural_log_exp_and_others":
            _fns.discard(AF.Exp)
            _fns.discard(AF.Ln)

    nc = bacc.Bacc("TRN2", target_bir_lowering=False, num_devices=num_devices)
    x_d = nc.dram_tensor("x", [B, C, N], f32, kind="ExternalInput").ap()
    wt_d = nc.dram_tensor("wt", [C, OW], bf16, kind="ExternalInput").ap()
    wot_d = nc.dram_tensor("wot", [4 * P, C], bf16, kind="ExternalInput").ap()
    xbf_d = nc.dram_tensor("xbf", [B, C, N], bf16, kind="ExternalInput").ap()
    cst_d = nc.dram_tensor("cst", [P, 16], f32, kind="ExternalInput").ap()
    cst4_d = nc.dram_tensor("cst4", [HEADS, 2 * P], f32, kind="ExternalInput").ap()
    out_d = nc.dram_tensor("out", [B, C, N], f32, kind="ExternalOutput").ap()

    with tile.TileContext(nc) as tc, ExitStack() as ctx:
        const = ctx.enter_context(tc.tile_pool(name="const", bufs=1))
        big = ctx.enter_context(tc.tile_pool(name="big", bufs=1))
        tmp = ctx.enter_context(tc.tile_pool(name="tmp", bufs=2))
        psA = ctx.enter_context(tc.tile_pool(name="psA", bufs=3, space="PSUM"))
        psB = ctx.enter_context(tc.tile_pool(name="psB", bufs=1, space="PSUM"))

        def mm(out, lhsT, rhs, start, stop, **kw):
            nc.tensor.matmul(out, lhsT, rhs, start=start, stop=stop, **kw)

        # ---------------- constants ----------------
        cst_f = tmp.tile([P, 16], f32, tag="cst_f", name="cst_f")
        nc.sync.dma_start(cst_f, cst_d[:])
        cst = const.tile([P, 16], bf16, tag="cst", name="cst")
        nc.vector.tensor_copy(out=cst[:], in_=cst_f[:])
        E_ind = [cst[:, 0:4], cst[:, 4:8]]
        rhs_x = cst[:, 8:10]
        rhs_q = cst[:, 10:12]
        negones = cst[:, 12:13]
        khalf = [cst[:, 13:14], cst[:, 15:16]]
        cst4_f = tmp.tile([HEADS, 2 * P], f32, tag="cst4_f", name="cst4_f")
        nc.sync.dma_start(cst4_f, cst4_d[:])
        cst4 = const.tile([HEADS, 2 * P], bf16, tag="cst4", name="cst4")
        nc.vector.tensor_copy(out=cst4[:], in_=cst4_f[:])
        E4 = [cst4[:, 0:128], cst4[:, 128:256]]

        ones2 = const.tile([P, D], bf16, tag="ones2", name="ones2")
        nc.vector.memset(ones2[:], 1.0)
        ones_row = const.tile([1, P], bf16, tag="ones_row", name="ones_row")
        nc.vector.memset(ones_row[:], 1.0)
        eps_col = const.tile([P, 1], f32, tag="eps_col", name="eps_col")
        nc.vector.memset(eps_col[:], LN_EPS)
        log8_col = const.tile([P, 1], f32, tag="log8_col", name="log8_col")
        nc.vector.memset(log8_col[:], LOG8)
        log8l_col = const.tile([P, 1], f32, tag="log8l_col", name="log8l_col")
        nc.vector.memset(log8l_col[:], LOG8L)

        # ---------------- PE warm-up ----------------
        # TRN2 PE DVFS ramps 0.65->1.2->2.4GHz only after ~3us of continuous
        # execution; idle resets it.  The first ~12us are DMA-bound with the
        # PE idle, so burn dummy matmuls there to enter the real work hot.
        warm = const.tile([P, 512], bf16, tag="warm", name="warm")
        nc.vector.memset(warm[:], 1.0)

        def _dummy_mm(n=1):
            for _ in range(n):
                wp = psA.tile([D, 512], f32, tag="A", name="wp")
                mm(wp[:], ones2[:, 0:D], warm[:], True, True)

        _dummy_mm(30)

        # ---------------- loads ----------------
        # xbf tiles first so stats can start ~2us in; x f32 loads deferred
        x_sb = [[big.tile([P, N], f32, tag=f"x{b}{cc}", name=f"x{b}{cc}")
                 for cc in range(2)] for b in range(B)]
        x_bf = [[big.tile([P, N], bf16, tag=f"xbf{b}{cc}", name=f"xbf{b}{cc}")
                 for cc in range(2)] for b in range(B)]
        xsq = [[big.tile([P, N], bf16, tag=f"xsq{b}{cc}", name=f"xsq{b}{cc}")
                for cc in range(2)] for b in range(B)]
        for b in range(B):
            for cc in range(2):
                nc.sync.dma_start(x_bf[b][cc], xbf_d[b, cc * P:(cc + 1) * P, :])
                nc.vector.tensor_mul(xsq[b][cc][:], x_bf[b][cc][:], x_bf[b][cc][:])

        wg_sb = big.tile([P, 2, OW], bf16, tag="wg", name="wg")
        nc.sync.dma_start(wg_sb, wt_d.rearrange("(cc p) o -> p cc o", p=P))
        wot_h = []
        for h in range(HEADS):
            wb = big.tile([P, C], bf16, tag=f"wot{h}", name=f"wot{h}")
            nc.sync.dma_start(wb, wot_d[h * P:(h + 1) * P, :])
            wot_h.append(wb)

        # ---------------- LN stats + centering (per batch) ----------------
        logrstd = [big.tile([P, 8], f32, tag=f"lrs{b}", name=f"lrs{b}") for b in range(B)]
        rstd_sb = [big.tile([P, 8], f32, tag=f"rstd{b}", name=f"rstd{b}") for b in range(B)]
        negmean_row = [big.tile([1, N], bf16, tag=f"nmr{b}", name=f"nmr{b}") for b in range(B)]

        def _stats_phase(b):
            st_ps = psA.tile([P, 8, 2], f32, tag="A", name="st_ps")
            for ic in range(8):
                sl = st_ps[:, ic]
                mm(sl, x_bf[b][0][:, ic * P:(ic + 1) * P], rhs_x, True, False)
                mm(sl, x_bf[b][1][:, ic * P:(ic + 1) * P], rhs_x, False, False)
                mm(sl, xsq[b][0][:, ic * P:(ic + 1) * P], rhs_q, False, False)
                mm(sl, xsq[b][1][:, ic * P:(ic + 1) * P], rhs_q, False, True)
            st_sb = tmp.tile([P, 8, 2], f32, tag="st_sb", name="st_sb")
            nc.vector.tensor_copy(out=st_sb[:], in_=st_ps[:])
            mean_v = st_sb[:, :, 0]
            msq_v = st_sb[:, :, 1]
            m2 = tmp.tile([P, 8], f32, tag="m2", name="m2")
            nc.vector.tensor_mul(m2[:], mean_v, mean_v)
            var = tmp.tile([P, 8], f32, tag="var", name="var")
            nc.vector.tensor_sub(var[:], msq_v, m2[:])
            lnv = tmp.tile([P, 8], f32, tag="lnv", name="lnv")
            nc.scalar.activation(lnv[:], var[:], AF.Ln, bias=eps_col[:])
            nc.vector.tensor_scalar_mul(logrstd[b][:], lnv[:], -0.5)
            nc.scalar.activation(rstd_sb[b][:], logrstd[b][:], AF.Exp)
            # negmean row + broadcast + center x in-place
            for ih in range(2):
                io = ih * 512
                nm_ps = psA.tile([1, 512], f32, tag="A", name="nm_ps")
                for cc in range(2):
                    mm(nm_ps[:], negones, x_bf[b][cc][:, io:io + 512],
                       start=(cc == 0), stop=(cc == 1))
                nc.vector.tensor_copy(out=negmean_row[b][:, io:io + 512], in_=nm_ps[:])
                nmbc_ps = psA.tile([P, 512], f32, tag="A", name="nmbc_ps")
                mm(nmbc_ps[:], ones_row[:],
                   negmean_row[b][:, io:io + 512], True, True)
                for cc in range(2):
                    nc.vector.tensor_add(x_bf[b][cc][:, io:io + 512],
                                         x_bf[b][cc][:, io:io + 512], nmbc_ps[:])

        # ---------------- QKV GEMM (q + padded k o-tiles) ----------------
        qkv_sb = [[big.tile([P, N], bf16, tag=f"qkv{b}{ot}", name=f"qkv{b}{ot}")
                   for ot in range(NQKV)] for b in range(B)]

        def _qkv_tile(b, ot, engine="v"):
            qk_ps = psA.tile([P, N], f32, tag="A", name="qk_ps")
            for ih in range(2):
                io = ih * 512
                mm(qk_ps[:, io:io + 512], wg_sb[:, 0, ot * P:(ot + 1) * P],
                   x_bf[b][0][:, io:io + 512], True, False)
                mm(qk_ps[:, io:io + 512], wg_sb[:, 1, ot * P:(ot + 1) * P],
                   x_bf[b][1][:, io:io + 512], False, True)
            if engine == "v":
                nc.vector.tensor_copy(out=qkv_sb[b][ot][:], in_=qk_ps[:])
            else:
                nc.scalar.copy(out=qkv_sb[b][ot][:], in_=qk_ps[:])

        # ---------------- v^T GEMM + augment (std col) ----------------
        # vaug[b]: [P(j), h, pair, sub, VW] e4m3; cols 0-63 v^T, 64 std,
        # 65-79 zero pad (DoubleRow LDWEIGHTS needs 16B-aligned subtile stride)
        VW = 80
        vaug = [big.tile([P, HEADS, 4, 2, VW], f8e4, tag=f"va{b}",
                         name=f"va{b}") for b in range(B)]
        for b in range(B):
            nc.gpsimd.memset(vaug[b][:, :, :, :, D:D + 1], 1.0)
            nc.gpsimd.memset(vaug[b][:, :, :, :, D + 1:VW], 0.0)

        def _vT_jc(b, jc):
            v_ps = psA.tile([P, 256], f32, tag="A", name="v_ps")
            for cc in range(2):
                mm(v_ps[:], x_bf[b][cc][:, jc * P:(jc + 1) * P],
                   wg_sb[:, cc, 768:1024], start=(cc == 0), stop=(cc == 1))
            p, s = jc // 2, jc % 2
            nc.vector.tensor_scalar_mul(
                vaug[b][:, :, p, s, 0:D],
                v_ps.rearrange("p (h d) -> p h d", h=HEADS),
                rstd_sb[b][:, jc:jc + 1])

        # ---------------- q/k norms ----------------
        b8_sb = [big.tile([P, 8, HEADS], f32, tag=f"b8{b}", name=f"b8{b}") for b in range(B)]
        b8s_sb = [big.tile([P, 8, HEADS], f32, tag=f"b8s{b}", name=f"b8s{b}") for b in range(B)]
        a_sb = [tmp.tile([HEADS, N], bf16, tag="a_sb", name=f"a_sb{b}") for b in range(B)]

        def _norm_phase(b):
            qsq = [tmp.tile([P, N], bf16, tag=f"qsq{cc}", name=f"qsq{cc}") for cc in range(2)]
            for cc in range(2):
                nc.vector.tensor_mul(qsq[cc][:], qkv_sb[b][cc][:], qkv_sb[b][cc][:])
            ksq = [tmp.tile([P, N], bf16, tag=f"ksq{h}", name=f"ksq{h}") for h in range(HEADS)]
            for h in range(HEADS):
                nc.vector.tensor_mul(ksq[h][:], qkv_sb[b][2 + h][:],
                                     qkv_sb[b][2 + h][:])
            a_ln = tmp.tile([HEADS, N], f32, tag="a_ln", name="a_ln")
            for ih in range(2):
                io = ih * 512
                s2q_ps = psA.tile([HEADS, 512], f32, tag="A", name="s2q_ps")
                for cc in range(2):
                    mm(s2q_ps[:], E_ind[cc], qsq[cc][:, io:io + 512],
                       start=(cc == 0), stop=(cc == 1))
                nc.scalar.activation(a_ln[:, io:io + 512], s2q_ps[:], AF.Ln)
            nc.scalar.activation(a_sb[b][:], a_ln[:], AF.Exp, scale=-0.5)
            bsq_ps = psA.tile([P, 8, HEADS], f32, tag="A", name="bsq_ps")
            for jc in range(8):
                for h in range(HEADS):
                    mm(bsq_ps[:, jc, h:h + 1], ksq[h][:, jc * P:(jc + 1) * P],
                       khalf[h % 2], True, True)
            b8ln = tmp.tile([P, 8, HEADS], f32, tag="b8ln", name="b8ln")
            nc.scalar.activation(b8ln[:], bsq_ps[:], AF.Ln)
            nc.scalar.activation(b8_sb[b][:], b8ln[:], AF.Exp, scale=-0.5,
                                 bias=log8_col[:])
            nc.scalar.activation(b8s_sb[b][:], b8ln[:], AF.Exp, scale=-0.5,
                                 bias=log8l_col[:])
            for cc in range(2):
                for ih in range(2):
                    io = ih * 512
                    abc_ps = psA.tile([P, 512], f32, tag="A", name="abc_ps")
                    mm(abc_ps[:], E4[cc], a_sb[b][:, io:io + 512], True, True)
                    nc.vector.tensor_mul(qkv_sb[b][cc][:, io:io + 512],
                                         qkv_sb[b][cc][:, io:io + 512], abc_ps[:])

        # ---------------- attention ----------------
        # et2 per head: [P(j), pair, sub, N(i)] u8 (== e5m2 bits)
        expp = ctx.enter_context(tc.tile_pool(name="expp", bufs=3))
        u65 = [[big.tile([D + 1, N], bf16, tag=f"u{b}{h}", name=f"u{b}{h}")
                for h in range(HEADS)] for b in range(B)]

        filler = []  # deferred closures: one popped after each attnV pair

        def _pop_filler(k=1):
            for _ in range(k):
                if filler:
                    filler.pop(0)()

        def _att_head(b, h):
            cc, hh = h // 2, h % 2
            dve_jcs = DVE_JCS[h]
            et2 = expp.tile([P, 4, 2, N], u8, tag="et2", name=f"et2_{b}{h}")
            et2_e5 = et2.bitcast(mybir.dt.float8e5)
            U_ps = psB.tile([VW, 2, 512], f32, tag="B", name="U_ps")

            def _attnV(p):
                for ih in range(2):
                    mm(U_ps[:, ih, :], vaug[b][:, h, p, :, :],
                       et2_e5[:, p, :, ih * 512:(ih + 1) * 512],
                       start=(p == 0), stop=(p == 3), perf_mode=DR)
                _pop_filler(1)

            for jc in range(8):
                p, s = jc // 2, jc % 2
                sim_ps = psA.tile([P, N], f32, tag="A", name="sim")
                for ih in range(2):
                    mm(sim_ps[:, ih * 512:(ih + 1) * 512],
                       qkv_sb[b][2 + h][:, jc * P:(jc + 1) * P],
                       qkv_sb[b][cc][:, ih * 512:(ih + 1) * 512],
                       True, True)
                if jc in dve_jcs:
                    nc.vector.tensor_scalar(
                        et2[:, p, s, :], sim_ps[:],
                        b8s_sb[b][:, jc, h:h + 1], 60.5,
                        op0=OP.mult, op1=OP.add)
                else:
                    nc.scalar.activation(et2_e5[:, p, s, :], sim_ps[:], AF.Exp,
                                         scale=b8_sb[b][:, jc, h:h + 1])
                # defer attnV of pair p until sims of pair p+1 are issued, so
                # the in-order PE queue never stalls waiting on the exp
                if s == 1 and p >= 1:
                    _attnV(p - 1)
            _attnV(3)
            nc.vector.tensor_copy(
                out=u65[b][h].rearrange("p (i n) -> p i n", i=2),
                in_=U_ps[0:D + 1, :, :])

        def _epi_h(b, h):
            for ih in range(2):
                io = ih * 512
                sbc_ps = psA.tile([D, 512], f32, tag="A", name="sbc_ps")
                mm(sbc_ps[:], ones2[D:D + 1, :],
                   u65[b][h][D:D + 1, io:io + 512], True, True)
                rbc = tmp.tile([D, 512], f32, tag="rbc", name="rbc")
                nc.vector.reciprocal_approx_fast(out=rbc[:], in_=sbc_ps[:])
                nc.gpsimd.tensor_mul(u65[b][h][0:D, io:io + 512],
                                     u65[b][h][0:D, io:io + 512], rbc[:])

        def _proj(b):
            for co in range(2):
                out_f = tmp.tile([P, N], f32, tag="out_f", name="out_f")
                for ih in range(2):
                    io = ih * 512
                    out_ps = psA.tile([P, 512], f32, tag="A", name="out_ps")
                    for h in range(HEADS):
                        mm(out_ps[:], wot_h[h][0:D, co * P:(co + 1) * P],
                           u65[b][h][0:D, io:io + 512],
                           start=(h == 0), stop=(h == 3))
                    nc.vector.tensor_add(out_f[:, io:io + 512],
                                         out_ps[:], x_sb[b][co][:, io:io + 512])
                nc.sync.dma_start(out_d[b, co * P:(co + 1) * P, :], out_f[:])

        # ---------------- schedule ----------------
        _stats_phase(0)
        for ot in (2, 3, 4, 5, 0, 1):
            _qkv_tile(0, ot)
        for jc in range(8):
            _vT_jc(0, jc)
        _norm_phase(0)
        for b in range(B):
            for cc in range(2):
                nc.sync.dma_start(x_sb[b][cc], x_d[b, cc * P:(cc + 1) * P, :])

        # batch-1 prep runs as filler inside batch-0's attention heads
        filler.append(lambda: _stats_phase(1))
        for ot in (2, 3):
            filler.append(lambda ot=ot: _qkv_tile(1, ot, engine="s"))
        for ot in (4, 5):
            filler.append(lambda ot=ot: _qkv_tile(1, ot))
        for jc in range(0, 8, 2):
            filler.append(lambda jc=jc: (_vT_jc(1, jc), _vT_jc(1, jc + 1)))
        for ot in (0, 1):
            filler.append(lambda ot=ot: _qkv_tile(1, ot))
        filler.append(lambda: _norm_phase(1))

        for h in range(HEADS):
            _att_head(0, h)
        _pop_filler(16)
        _att_head(1, 0)
        _epi_h(0, 0)
        _att_head(1, 1)
        _epi_h(0, 1)
        _epi_h(1, 0)
        _att_head(1, 2)
        _epi_h(0, 2)
        _epi_h(1, 1)
        _att_head(1, 3)
        _epi_h(0, 3)
        _proj(0)
        _epi_h(1, 2)
        _epi_h(1, 3)
        _proj(1)

    nc.compile()
    return nc


_NC = None
last_exec_time_ns = None


def _get_nc():
    global _NC
    if _NC is None:
        _NC = build_nc()
    return _NC


def _run(in_maps, trace=False):
    global last_exec_time_ns
    from concourse.bass_utils import run_bass_kernel_spmd
    nc = _get_nc()
    if trace:
        _install_ntff_hook()
    try:
        res = run_bass_kernel_spmd(nc, in_maps, core_ids=list(range(NCORES)),
                                   trace=trace)
    except Exception:
        if not trace:
            raise
        res = run_bass_kernel_spmd(nc, in_maps, core_ids=list(range(NCORES)),
                                   trace=False)
    last_exec_time_ns = res.exec_time_ns
    return res


def make_in_maps(x, g, w_qkv, w_out, ncores=NCORES):
    import ml_dtypes as _md
    x = np.ascontiguousarray(np.asarray(x, dtype=np.float32))
    g = np.asarray(g, dtype=np.float32).reshape(C)
    wt_pad, wot_pad = _host_weights(w_qkv, w_out, g)
    b_full = x.shape[0]
    xr = x.reshape(b_full, C, N)
    cst, cst4 = _host_consts()
    in_maps = []
    for i in range(ncores):
        in_maps.append({
            "x": np.ascontiguousarray(xr[i * B:(i + 1) * B]),
            "xbf": np.ascontiguousarray(xr[i * B:(i + 1) * B]).astype(_md.bfloat16),
            "wt": wt_pad,
            "wot": wot_pad,
            "cst": cst,
            "cst4": cst4,
        })
    return in_maps


def kernel(x, g, w_qkv, w_out, _trace=False):
    x = np.ascontiguousarray(np.asarray(x, dtype=np.float32))
    b_full, c, H, W = x.shape
    assert (b_full, c, H * W) == (NCORES * B, C, N)
    in_maps = make_in_maps(x, g, w_qkv, w_out)
    res = _run(in_maps, trace=_trace)
    out = np.concatenate([res.results[i]["out"] for i in range(NCORES)], axis=0)
    return out.reshape(b_full, C, H, W).astype(np.float32)


# revision 23
# speedup vs baseline: 1.1926x; 1.1926x over previous
"""Trainium2 Bass kernel for nn_Attention_730144440595 (NormAttention block).

8 NeuronCores, data-parallel over batch (16 -> 2/core). Per core:
  - channel-LN folded: x centered on-device (mean via PE ones-matmuls, broadcast
    via K=1 matmul); per-pixel rstd folded into the softmax exp as a log-bias.
  - QKV GEMM produces q (2 o-tiles) and zero-padded per-head k (4 o-tiles);
    v is produced TRANSPOSED (v^T[j, h*64+d]) by a second small GEMM so the
    attn@V contraction needs no DMA transposes.
  - q L2-norm applied via PE-broadcast multiply; k-norm and the x8 scale fold
    into the softmax exp's per-partition scale, per-pixel rstd into its bias.
  - attention transposed (sim^T[j,i]); exp emitted directly in fp8-e5m2,
    most on ACT (native Exp) and some on DVE (Schraudolph bit-trick:
    u8 = trunc(logit*4*log2(e) + 60.5) reinterpreted as e5m2).
  - attn@V in fp8 DoubleRow mode (K=256 per instruction over jc pairs) with
    e4m3 v^T augmented by a std column so the softmax denominator drops out
    of the same matmul.  Out-proj in bf16 (K=64/head), f32 PSUM everywhere.
"""

import sys
import types

import numpy as np

B = 2
C = 256
N = 1024
HEADS = 4
D = 64
P = 128
NCORES = 8
LN_EPS = 1e-5
LOG8 = float(np.log(8.0))
LOG2E4 = float(4.0 / np.log(2.0))          # 5.770780163555851
LOG8L = float(np.log(8.0 * 4.0 / np.log(2.0)))  # log(8 * 4*log2(e))
NQKV = 6         # qkv o-tiles: q 0-1, k(padded) 2-5
OW = 1024        # weight cols: q 0-255, k-pad 256-767, vT 768-1023
# which jc's exp runs on DVE (bit-trick) instead of ACT, per head
DVE_JCS = ((1, 3, 6), (2, 4, 7), (1, 3, 6), (2, 4, 7))


def _host_consts():
    cst = np.zeros((P, 16), np.float32)
    # E_ind[cc][p, h] = 1 iff h == 2*cc + p//64   (cols 0-3 / 4-7)
    for cc in range(2):
        for p in range(P):
            cst[p, 4 * cc + 2 * cc + p // 64] = 1.0
    cst[:, 8] = 1.0 / 256.0    # rhs_x col0 (mean)
    cst[:, 11] = 1.0 / 256.0   # rhs_q col1 (msq)  (cols 10-11)
    cst[:, 12] = -1.0 / 256.0  # negones
    cst[:64, 13] = 1.0         # khalf indicator (rows 0-63)
    cst[:, 14] = 1.0           # ones col
    cst[64:, 15] = 1.0         # khalf_odd (rows 64-127)
    cst4 = np.zeros((HEADS, 2 * P), np.float32)
    for cc in range(2):
        for m in range(P):
            cst4[2 * cc + m // 64, cc * P + m] = 1.0
    return cst, cst4


def _host_weights(w_qkv, w_out, g):
    """wt_pad [C, 1024]: q cols 0-255, k head h in its own zero-padded 128-col
    block (256+128h..), vT cols 768-1023 = w_v^T laid out (c, h*64+d).
    g folded in on host.  wot_pad [512, C]: head h rows 128h..128h+64."""
    import ml_dtypes
    wg = np.asarray(w_qkv, np.float32) * np.asarray(g, np.float32).reshape(1, C)
    wt = np.ascontiguousarray(wg.T)  # [C, 768]
    wt_pad = np.zeros((C, OW), np.float32)
    wt_pad[:, 0:256] = wt[:, 0:256]
    for h in range(HEADS):
        off = 256 + 128 * h + (h % 2) * D
        wt_pad[:, off:off + D] = wt[:, 256 + D * h:256 + D * (h + 1)]
    wt_pad[:, 768:1024] = wt[:, 512:768]
    wot = np.ascontiguousarray(np.asarray(w_out, np.float32).T)  # [ci, co]
    wot_pad = np.zeros((4 * P, C), np.float32)
    for h in range(HEADS):
        wot_pad[128 * h:128 * h + D, :] = wot[D * h:D * (h + 1), :]
    return wt_pad.astype(ml_dtypes.bfloat16), wot_pad.astype(ml_dtypes.bfloat16)


def _install_ntff_hook():
    try:
        import antenv
        if getattr(antenv, "axon_hooks", None) is not None:
            return
        from trn_agent_boot.trn_boot import _ntff_profile_via_ctypes
        hook = _ntff_profile_via_ctypes('/opt/axon/libaxon_pjrt.so')
        mod = types.ModuleType('antenv.axon_hooks')
        mod._hook = hook
        mod.get_axon_ntff_profile_hook = lambda: mod._hook
        mod.set_axon_ntff_profile_hook = lambda h: setattr(mod, '_hook', h)
        sys.modules['antenv.axon_hooks'] = mod
        antenv.axon_hooks = mod
    except Exception:
        pass


def build_nc(num_devices=NCORES):
    import concourse.bass as bass
    import concourse.tile as tile
    import concourse.mybir as mybir
    from concourse import bacc
    from contextlib import ExitStack

    dt = mybir.dt
    f32 = dt.float32
    bf16 = dt.bfloat16
    f8e4 = dt.float8e4
    f8e5 = dt.float8e5
    u8 = dt.uint8
    AF = mybir.ActivationFunctionType
    OP = mybir.AluOpType
    DR = mybir.MatmulPerfMode.DoubleRow

    # Steer the ACT table selector: keep Exp/Ln only in the combined set so
    # the kernel never thrashes between exp_and_others and natural_log sets
    # (each switch costs ~2.6us of ACT_TABLE_LOAD + drain).
    from concourse.hw_specs import get_activation_tables
    _tabs = get_activation_tables("gen3")
    for _name, _fns in _tabs.items():
        if _name != "natural_log_exp_and_others":
            _fns.discard(AF.Exp)
            _fns.discard(AF.Ln)

    nc = bacc.Bacc("TRN2", target_bir_lowering=False, num_devices=num_devices)
    x_d = nc.dram_tensor("x", [B, C, N], f32, kind="ExternalInput").ap()
    wt_d = nc.dram_tensor("wt", [C, OW], bf16, kind="ExternalInput").ap()
    wot_d = nc.dram_tensor("wot", [4 * P, C], bf16, kind="ExternalInput").ap()
    xbf_d = nc.dram_tensor("xbf", [B, C, N], bf16, kind="ExternalInput").ap()
    cst_d = nc.dram_tensor("cst", [P, 16], f32, kind="ExternalInput").ap()
    cst4_d = nc.dram_tensor("cst4", [HEADS, 2 * P], f32, kind="ExternalInput").ap()
    out_d = nc.dram_tensor("out", [B, C, N], f32, kind="ExternalOutput").ap()

    with tile.TileContext(nc) as tc, ExitStack() as ctx:
        const = ctx.enter_context(tc.tile_pool(name="const", bufs=1))
        big = ctx.enter_context(tc.tile_pool(name="big", bufs=1))
        tmp = ctx.enter_context(tc.tile_pool(name="tmp", bufs=2))
        psA = ctx.enter_context(tc.tile_pool(name="psA", bufs=3, space="PSUM"))
        psB = ctx.enter_context(tc.tile_pool(name="psB", bufs=1, space="PSUM"))

        def mm(out, lhsT, rhs, start, stop, **kw):
            nc.tensor.matmul(out, lhsT, rhs, start=start, stop=stop, **kw)

        # ---------------- constants ----------------
        cst_f = tmp.tile([P, 16], f32, tag="cst_f", name="cst_f")
        nc.sync.dma_start(cst_f, cst_d[:])
        cst = const.tile([P, 16], bf16, tag="cst", name="cst")
        nc.vector.tensor_copy(out=cst[:], in_=cst_f[:])
        E_ind = [cst[:, 0:4], cst[:, 4:8]]
        rhs_x = cst[:, 8:10]
        rhs_q = cst[:, 10:12]
        negones = cst[:, 12:13]
        khalf = [cst[:, 13:14], cst[:, 15:16]]
        cst4_f = tmp.tile([HEADS, 2 * P], f32, tag="cst4_f", name="cst4_f")
        nc.sync.dma_start(cst4_f, cst4_d[:])
        cst4 = const.tile([HEADS, 2 * P], bf16, tag="cst4", name="cst4")
        nc.vector.tensor_copy(out=cst4[:], in_=cst4_f[:])
        E4 = [cst4[:, 0:128], cst4[:, 128:256]]

        ones2 = const.tile([P, D], bf16, tag="ones2", name="ones2")
        nc.vector.memset(ones2[:], 1.0)
        ones_row = const.tile([1, P], bf16, tag="ones_row", name="ones_row")
        nc.vector.memset(ones_row[:], 1.0)
        eps_col = const.tile([P, 1], f32, tag="eps_col", name="eps_col")
        nc.vector.memset(eps_col[:], LN_EPS)
        log8_col = const.tile([P, 1], f32, tag="log8_col", name="log8_col")
        nc.vector.memset(log8_col[:], LOG8)
        log8l_col = const.tile([P, 1], f32, tag="log8l_col", name="log8l_col")
        nc.vector.memset(log8l_col[:], LOG8L)

        # ---------------- PE warm-up ----------------
        # TRN2 PE DVFS ramps 0.65->1.2->2.4GHz only after ~3us of continuous
        # execution; idle resets it.  The first ~12us are DMA-bound with the
        # PE idle, so burn dummy matmuls there to enter the real work hot.
        warm = const.tile([P, 512], bf16, tag="warm", name="warm")
        nc.vector.memset(warm[:], 1.0)

        def _dummy_mm(n=1):
            for _ in range(n):
                wp = psA.tile([D, 512], f32, tag="A", name="wp")
                mm(wp[:], ones2[:, 0:D], warm[:], True, True)

        _dummy_mm(30)

        # ---------------- loads ----------------
        # xbf tiles first so stats can start ~2us in; x f32 loads deferred
        x_sb = [[big.tile([P, N], f32, tag=f"x{b}{cc}", name=f"x{b}{cc}")
                 for cc in range(2)] for b in range(B)]
        x_bf = [[big.tile([P, N], bf16, tag=f"xbf{b}{cc}", name=f"xbf{b}{cc}")
                 for cc in range(2)] for b in range(B)]
        xsq = [[big.tile([P, N], bf16, tag=f"xsq{b}{cc}", name=f"xsq{b}{cc}")
                for cc in range(2)] for b in range(B)]
        for b in range(B):
            for cc in range(2):
                nc.sync.dma_start(x_bf[b][cc], xbf_d[b, cc * P:(cc + 1) * P, :])
                nc.vector.tensor_mul(xsq[b][cc][:], x_bf[b][cc][:], x_bf[b][cc][:])

        wg_sb = big.tile([P, 2, OW], bf16, tag="wg", name="wg")
        nc.sync.dma_start(wg_sb, wt_d.rearrange("(cc p) o -> p cc o", p=P))
        wot_h = []
        for h in range(HEADS):
            wb = big.tile([P, C], bf16, tag=f"wot{h}", name=f"wot{h}")
            nc.sync.dma_start(wb, wot_d[h * P:(h + 1) * P, :])
            wot_h.append(wb)

        # ---------------- LN stats + centering (per batch) ----------------
        logrstd = [big.tile([P, 8], f32, tag=f"lrs{b}", name=f"lrs{b}") for b in range(B)]
        rstd_sb = [big.tile([P, 8], f32, tag=f"rstd{b}", name=f"rstd{b}") for b in range(B)]
        negmean_row = [big.tile([1, N], bf16, tag=f"nmr{b}", name=f"nmr{b}") for b in range(B)]

        def _stats_phase(b):
            st_ps = psA.tile([P, 8, 2], f32, tag="A", name="st_ps")
            for ic in range(8):
                sl = st_ps[:, ic]
                mm(sl, x_bf[b][0][:, ic * P:(ic + 1) * P], rhs_x, True, False)
                mm(sl, x_bf[b][1][:, ic * P:(ic + 1) * P], rhs_x, False, False)
                mm(sl, xsq[b][0][:, ic * P:(ic + 1) * P], rhs_q, False, False)
                mm(sl, xsq[b][1][:, ic * P:(ic + 1) * P], rhs_q, False, True)
            st_sb = tmp.tile([P, 8, 2], f32, tag="st_sb", name="st_sb")
            nc.vector.tensor_copy(out=st_sb[:], in_=st_ps[:])
            mean_v = st_sb[:, :, 0]
            msq_v = st_sb[:, :, 1]
            m2 = tmp.tile([P, 8], f32, tag="m2", name="m2")
            nc.vector.tensor_mul(m2[:], mean_v, mean_v)
            var = tmp.tile([P, 8], f32, tag="var", name="var")
            nc.vector.tensor_sub(var[:], msq_v, m2[:])
            lnv = tmp.tile([P, 8], f32, tag="lnv", name="lnv")
            nc.scalar.activation(lnv[:], var[:], AF.Ln, bias=eps_col[:])
            nc.vector.tensor_scalar_mul(logrstd[b][:], lnv[:], -0.5)
            nc.scalar.activation(rstd_sb[b][:], logrstd[b][:], AF.Exp)
            # negmean row + broadcast + center x in-place
            for ih in range(2):
                io = ih * 512
                nm_ps = psA.tile([1, 512], f32, tag="A", name="nm_ps")
                for cc in range(2):
                    mm(nm_ps[:], negones, x_bf[b][cc][:, io:io + 512],
                       start=(cc == 0), stop=(cc == 1))
                nc.vector.tensor_copy(out=negmean_row[b][:, io:io + 512], in_=nm_ps[:])
                nmbc_ps = psA.tile([P, 512], f32, tag="A", name="nmbc_ps")
                mm(nmbc_ps[:], ones_row[:],
                   negmean_row[b][:, io:io + 512], True, True)
                for cc in range(2):
                    nc.vector.tensor_add(x_bf[b][cc][:, io:io + 512],
                                         x_bf[b][cc][:, io:io + 512], nmbc_ps[:])

        # ---------------- QKV GEMM (q + padded k o-tiles) ----------------
        qkv_sb = [[big.tile([P, N], bf16, tag=f"qkv{b}{ot}", name=f"qkv{b}{ot}")
                   for ot in range(NQKV)] for b in range(B)]

        def _qkv_tile(b, ot, engine="v"):
            qk_ps = psA.tile([P, N], f32, tag="A", name="qk_ps")
            for ih in range(2):
                io = ih * 512
                mm(qk_ps[:, io:io + 512], wg_sb[:, 0, ot * P:(ot + 1) * P],
                   x_bf[b][0][:, io:io + 512], True, False)
                mm(qk_ps[:, io:io + 512], wg_sb[:, 1, ot * P:(ot + 1) * P],
                   x_bf[b][1][:, io:io + 512], False, True)
            if engine == "v":
                nc.vector.tensor_copy(out=qkv_sb[b][ot][:], in_=qk_ps[:])
            else:
                nc.scalar.copy(out=qkv_sb[b][ot][:], in_=qk_ps[:])

        # ---------------- v^T GEMM + augment (std col) ----------------
        # vaug[b]: [P(j), h, pair, sub, VW] e4m3; cols 0-63 v^T, 64 std,
        # 65-79 zero pad (DoubleRow LDWEIGHTS needs 16B-aligned subtile stride)
        VW = 80
        vaug = [big.tile([P, HEADS, 4, 2, VW], f8e4, tag=f"va{b}",
                         name=f"va{b}") for b in range(B)]
        for b in range(B):
            nc.gpsimd.memset(vaug[b][:, :, :, :, D:D + 1], 1.0)
            nc.gpsimd.memset(vaug[b][:, :, :, :, D + 1:VW], 0.0)

        def _vT_jc(b, jc):
            v_ps = psA.tile([P, 256], f32, tag="A", name="v_ps")
            for cc in range(2):
                mm(v_ps[:], x_bf[b][cc][:, jc * P:(jc + 1) * P],
                   wg_sb[:, cc, 768:1024], start=(cc == 0), stop=(cc == 1))
            p, s = jc // 2, jc % 2
            nc.vector.tensor_scalar_mul(
                vaug[b][:, :, p, s, 0:D],
                v_ps.rearrange("p (h d) -> p h d", h=HEADS),
                rstd_sb[b][:, jc:jc + 1])

        # ---------------- q/k norms ----------------
        b8_sb = [big.tile([P, 8, HEADS], f32, tag=f"b8{b}", name=f"b8{b}") for b in range(B)]
        b8s_sb = [big.tile([P, 8, HEADS], f32, tag=f"b8s{b}", name=f"b8s{b}") for b in range(B)]
        a_sb = [tmp.tile([HEADS, N], bf16, tag="a_sb", name=f"a_sb{b}") for b in range(B)]

        def _norm_phase(b):
            qsq = [tmp.tile([P, N], bf16, tag=f"qsq{cc}", name=f"qsq{cc}") for cc in range(2)]
            for cc in range(2):
                nc.vector.tensor_mul(qsq[cc][:], qkv_sb[b][cc][:], qkv_sb[b][cc][:])
            ksq = [tmp.tile([P, N], bf16, tag=f"ksq{h}", name=f"ksq{h}") for h in range(HEADS)]
            for h in range(HEADS):
                nc.vector.tensor_mul(ksq[h][:], qkv_sb[b][2 + h][:],
                                     qkv_sb[b][2 + h][:])
            a_ln = tmp.tile([HEADS, N], f32, tag="a_ln", name="a_ln")
            for ih in range(2):
                io = ih * 512
                s2q_ps = psA.tile([HEADS, 512], f32, tag="A", name="s2q_ps")
                for cc in range(2):
                    mm(s2q_ps[:], E_ind[cc], qsq[cc][:, io:io + 512],
                       start=(cc == 0), stop=(cc == 1))
                nc.scalar.activation(a_ln[:, io:io + 512], s2q_ps[:], AF.Ln)
            nc.scalar.activation(a_sb[b][:], a_ln[:], AF.Exp, scale=-0.5)
            bsq_ps = psA.tile([P, 8, HEADS], f32, tag="A", name="bsq_ps")
            for jc in range(8):
                for h in range(HEADS):
                    mm(bsq_ps[:, jc, h:h + 1], ksq[h][:, jc * P:(jc + 1) * P],
                       khalf[h % 2], True, True)
            b8ln = tmp.tile([P, 8, HEADS], f32, tag="b8ln", name="b8ln")
            nc.scalar.activation(b8ln[:], bsq_ps[:], AF.Ln)
            nc.scalar.activation(b8_sb[b][:], b8ln[:], AF.Exp, scale=-0.5,
                                 bias=log8_col[:])
            nc.scalar.activation(b8s_sb[b][:], b8ln[:], AF.Exp, scale=-0.5,
                                 bias=log8l_col[:])
            for cc in range(2):
                for ih in range(2):
                    io = ih * 512
                    abc_ps = psA.tile([P, 512], f32, tag="A", name="abc_ps")
                    mm(abc_ps[:], E4[cc], a_sb[b][:, io:io + 512], True, True)
                    nc.vector.tensor_mul(qkv_sb[b][cc][:, io:io + 512],
                                         qkv_sb[b][cc][:, io:io + 512], abc_ps[:])

        # ---------------- attention ----------------
        # et2 per head: [P(j), pair, sub, N(i)] u8 (== e5m2 bits)
        expp = ctx.enter_context(tc.tile_pool(name="expp", bufs=3))
        u65 = [[big.tile([D + 1, N], bf16, tag=f"u{b}{h}", name=f"u{b}{h}")
                for h in range(HEADS)] for b in range(B)]

        filler = []  # deferred closures: one popped after each attnV pair

        def _pop_filler(k=1):
            for _ in range(k):
                if filler:
                    filler.pop(0)()

        def _att_head(b, h):
            cc, hh = h // 2, h % 2
            dve_jcs = DVE_JCS[h]
            et2 = expp.tile([P, 4, 2, N], u8, tag="et2", name=f"et2_{b}{h}")
            et2_e5 = et2.bitcast(mybir.dt.float8e5)
            U_ps = psB.tile([VW, 2, 512], f32, tag="B", name="U_ps")

            def _attnV(p):
                for ih in range(2):
                    mm(U_ps[:, ih, :], vaug[b][:, h, p, :, :],
                       et2_e5[:, p, :, ih * 512:(ih + 1) * 512],
                       start=(p == 0), stop=(p == 3), perf_mode=DR)
                _pop_filler(1)

            for jc in range(8):
                p, s = jc // 2, jc % 2
                sim_ps = psA.tile([P, N], f32, tag="A", name="sim")
                for ih in range(2):
                    mm(sim_ps[:, ih * 512:(ih + 1) * 512],
                       qkv_sb[b][2 + h][:, jc * P:(jc + 1) * P],
                       qkv_sb[b][cc][:, ih * 512:(ih + 1) * 512],
                       True, True)
                if jc in dve_jcs:
                    nc.vector.tensor_scalar(
                        et2[:, p, s, :], sim_ps[:],
                        b8s_sb[b][:, jc, h:h + 1], 60.5,
                        op0=OP.mult, op1=OP.add)
                else:
                    nc.scalar.activation(et2_e5[:, p, s, :], sim_ps[:], AF.Exp,
                                         scale=b8_sb[b][:, jc, h:h + 1])
                # defer attnV of pair p until sims of pair p+1 are issued, so
                # the in-order PE queue never stalls waiting on the exp
                if s == 1 and p >= 1:
                    _attnV(p - 1)
            _attnV(3)
            nc.vector.tensor_copy(
                out=u65[b][h].rearrange("p (i n) -> p i n", i=2),
                in_=U_ps[0:D + 1, :, :])

        def _epi_h(b, h):
            for ih in range(2):
                io = ih * 512
                sbc_ps = psA.tile([D, 512], f32, tag="A", name="sbc_ps")
                mm(sbc_ps[:], ones2[D:D + 1, :],
                   u65[b][h][D:D + 1, io:io + 512], True, True)
                rbc = tmp.tile([D, 512], f32, tag="rbc", name="rbc")
                nc.vector.reciprocal_approx_fast(out=rbc[:], in_=sbc_ps[:])
                nc.gpsimd.tensor_mul(u65[b][h][0:D, io:io + 512],
                                     u65[b][h][0:D, io:io + 512], rbc[:])

        def _proj(b):
            for co in range(2):
                out_f = tmp.tile([P, N], f32, tag="out_f", name="out_f")
                for ih in range(2):
                    io = ih * 512
                    out_ps = psA.tile([P, 512], f32, tag="A", name="out_ps")
                    for h in range(HEADS):
                        mm(out_ps[:], wot_h[h][0:D, co * P:(co + 1) * P],
                           u65[b][h][0:D, io:io + 512],
                           start=(h == 0), stop=(h == 3))
                    nc.vector.tensor_add(out_f[:, io:io + 512],
                                         out_ps[:], x_sb[b][co][:, io:io + 512])
                nc.sync.dma_start(out_d[b, co * P:(co + 1) * P, :], out_f[:])

        # ---------------- schedule ----------------
        _stats_phase(0)
        for ot in (2, 3, 4, 5, 0, 1):
            _qkv_tile(0, ot)
        for jc in range(8):
            _vT_jc(0, jc)
        _norm_phase(0)
        for b in range(B):
            for cc in range(2):
                nc.sync.dma_start(x_sb[b][cc], x_d[b, cc * P:(cc + 1) * P, :])

        # batch-1 prep runs as filler inside batch-0's attention heads
        filler.append(lambda: _stats_phase(1))
        for ot in (2, 3):
            filler.append(lambda ot=ot: _qkv_tile(1, ot, engine="s"))
        for ot in (4, 5):
            filler.append(lambda ot=ot: _qkv_tile(1, ot))
        for jc in range(0, 8, 2):
            filler.append(lambda jc=jc: (_vT_jc(1, jc), _vT_jc(1, jc + 1)))
        for ot in (0, 1):
            filler.append(lambda ot=ot: _qkv_tile(1, ot))
        filler.append(lambda: _norm_phase(1))

        for h in range(HEADS):
            _att_head(0, h)
        _pop_filler(16)
        _att_head(1, 0)
        _epi_h(0, 0)
        _att_head(1, 1)
        _epi_h(0, 1)
        _epi_h(1, 0)
        _att_head(1, 2)
        _epi_h(0, 2)
        _epi_h(1, 1)
        _att_head(1, 3)
        _epi_h(0, 3)
        _proj(0)
        _epi_h(1, 2)
        _epi_h(1, 3)
        _proj(1)

    nc.compile()
    return nc


_NC = None
last_exec_time_ns = None


def _get_nc():
    global _NC
    if _NC is None:
        _NC = build_nc()
    return _NC


def _run(in_maps, trace=False):
    global last_exec_time_ns
    from concourse.bass_utils import run_bass_kernel_spmd
    nc = _get_nc()
    if trace:
        _install_ntff_hook()
    try:
        res = run_bass_kernel_spmd(nc, in_maps, core_ids=list(range(NCORES)),
                                   trace=trace)
    except Exception:
        if not trace:
            raise
        res = run_bass_kernel_spmd(nc, in_maps, core_ids=list(range(NCORES)),
                                   trace=False)
    last_exec_time_ns = res.exec_time_ns
    return res


def make_in_maps(x, g, w_qkv, w_out, ncores=NCORES):
    import ml_dtypes as _md
    x = np.ascontiguousarray(np.asarray(x, dtype=np.float32))
    g = np.asarray(g, dtype=np.float32).reshape(C)
    wt_pad, wot_pad = _host_weights(w_qkv, w_out, g)
    b_full = x.shape[0]
    xr = x.reshape(b_full, C, N)
    cst, cst4 = _host_consts()
    in_maps = []
    for i in range(ncores):
        in_maps.append({
            "x": np.ascontiguousarray(xr[i * B:(i + 1) * B]),
            "xbf": np.ascontiguousarray(xr[i * B:(i + 1) * B]).astype(_md.bfloat16),
            "wt": wt_pad,
            "wot": wot_pad,
            "cst": cst,
            "cst4": cst4,
        })
    return in_maps


def kernel(x, g, w_qkv, w_out, _trace=False):
    x = np.ascontiguousarray(np.asarray(x, dtype=np.float32))
    b_full, c, H, W = x.shape
    assert (b_full, c, H * W) == (NCORES * B, C, N)
    in_maps = make_in_maps(x, g, w_qkv, w_out)
    res = _run(in_maps, trace=_trace)
    out = np.concatenate([res.results[i]["out"] for i in range(NCORES)], axis=0)
    return out.reshape(b_full, C, H, W).astype(np.float32)


# revision 24
# speedup vs baseline: 1.2116x; 1.0159x over previous
"""Trainium2 Bass kernel for nn_Attention_730144440595 (NormAttention block).

8 NeuronCores, data-parallel over batch (16 -> 2/core). Per core:
  - channel-LN folded: x centered on-device (mean via PE ones-matmuls, broadcast
    via K=1 matmul); per-pixel rstd folded into the softmax exp as a log-bias.
  - QKV GEMM produces q (2 o-tiles) and zero-padded per-head k (4 o-tiles);
    v is produced TRANSPOSED (v^T[j, h*64+d]) by a second small GEMM so the
    attn@V contraction needs no DMA transposes.
  - q L2-norm applied via PE-broadcast multiply; k-norm and the x8 scale fold
    into the softmax exp's per-partition scale, per-pixel rstd into its bias.
  - attention transposed (sim^T[j,i]); exp emitted directly in fp8-e5m2,
    most on ACT (native Exp) and some on DVE (Schraudolph bit-trick:
    u8 = trunc(logit*4*log2(e) + 60.5) reinterpreted as e5m2).
  - attn@V in fp8 DoubleRow mode (K=256 per instruction over jc pairs) with
    e4m3 v^T augmented by a std column so the softmax denominator drops out
    of the same matmul.  Out-proj in bf16 (K=64/head), f32 PSUM everywhere.
"""

import sys
import types

import numpy as np

B = 2
C = 256
N = 1024
HEADS = 4
D = 64
P = 128
NCORES = 8
LN_EPS = 1e-5
LOG8 = float(np.log(8.0))
LOG2E4 = float(4.0 / np.log(2.0))          # 5.770780163555851
LOG8L = float(np.log(8.0 * 4.0 / np.log(2.0)))  # log(8 * 4*log2(e))
NQKV = 6         # qkv o-tiles: q 0-1, k(padded) 2-5
OW = 1024        # weight cols: q 0-255, k-pad 256-767, vT 768-1023
# which jc's exp runs on DVE (bit-trick) instead of ACT, per head
DVE_JCS = ((1, 3, 6), (2, 4, 7), (1, 3, 6), (2, 4, 7))


def _host_consts():
    cst = np.zeros((P, 16), np.float32)
    # E_ind[cc][p, h] = 1 iff h == 2*cc + p//64   (cols 0-3 / 4-7)
    for cc in range(2):
        for p in range(P):
            cst[p, 4 * cc + 2 * cc + p // 64] = 1.0
    cst[:, 8] = 1.0 / 256.0    # rhs_x col0 (mean)
    cst[:, 11] = 1.0 / 256.0   # rhs_q col1 (msq)  (cols 10-11)
    cst[:, 12] = -1.0 / 256.0  # negones
    cst[:64, 13] = 1.0         # khalf indicator (rows 0-63)
    cst[:, 14] = 1.0           # ones col
    cst[64:, 15] = 1.0         # khalf_odd (rows 64-127)
    cst4 = np.zeros((HEADS, 2 * P), np.float32)
    for cc in range(2):
        for m in range(P):
            cst4[2 * cc + m // 64, cc * P + m] = 1.0
    return cst, cst4


def _host_weights(w_qkv, w_out, g):
    """wt_pad [C, 1024]: q cols 0-255, k head h in its own zero-padded 128-col
    block (256+128h..), vT cols 768-1023 = w_v^T laid out (c, h*64+d).
    g folded in on host.  wot_pad [512, C]: head h rows 128h..128h+64."""
    import ml_dtypes
    wg = np.asarray(w_qkv, np.float32) * np.asarray(g, np.float32).reshape(1, C)
    wt = np.ascontiguousarray(wg.T)  # [C, 768]
    wt_pad = np.zeros((C, OW), np.float32)
    wt_pad[:, 0:256] = wt[:, 0:256]
    for h in range(HEADS):
        off = 256 + 128 * h + (h % 2) * D
        wt_pad[:, off:off + D] = wt[:, 256 + D * h:256 + D * (h + 1)]
    wt_pad[:, 768:1024] = wt[:, 512:768]
    wot = np.ascontiguousarray(np.asarray(w_out, np.float32).T)  # [ci, co]
    wot_pad = np.zeros((4 * P, C), np.float32)
    for h in range(HEADS):
        wot_pad[128 * h:128 * h + D, :] = wot[D * h:D * (h + 1), :]
    return wt_pad.astype(ml_dtypes.bfloat16), wot_pad.astype(ml_dtypes.bfloat16)


def _install_ntff_hook():
    try:
        import antenv
        if getattr(antenv, "axon_hooks", None) is not None:
            return
        from trn_agent_boot.trn_boot import _ntff_profile_via_ctypes
        hook = _ntff_profile_via_ctypes('/opt/axon/libaxon_pjrt.so')
        mod = types.ModuleType('antenv.axon_hooks')
        mod._hook = hook
        mod.get_axon_ntff_profile_hook = lambda: mod._hook
        mod.set_axon_ntff_profile_hook = lambda h: setattr(mod, '_hook', h)
        sys.modules['antenv.axon_hooks'] = mod
        antenv.axon_hooks = mod
    except Exception:
        pass


def build_nc(num_devices=NCORES):
    import concourse.bass as bass
    import concourse.tile as tile
    import concourse.mybir as mybir
    from concourse import bacc
    from contextlib import ExitStack

    dt = mybir.dt
    f32 = dt.float32
    bf16 = dt.bfloat16
    f8e4 = dt.float8e4
    f8e5 = dt.float8e5
    u8 = dt.uint8
    AF = mybir.ActivationFunctionType
    OP = mybir.AluOpType
    DR = mybir.MatmulPerfMode.DoubleRow

    # Steer the ACT table selector: keep Exp/Ln only in the combined set so
    # the kernel never thrashes between exp_and_others and natural_log sets
    # (each switch costs ~2.6us of ACT_TABLE_LOAD + drain).
    from concourse.hw_specs import get_activation_tables
    _tabs = get_activation_tables("gen3")
    for _name, _fns in _tabs.items():
        if _name != "natural_log_exp_and_others":
            _fns.discard(AF.Exp)
            _fns.discard(AF.Ln)

    nc = bacc.Bacc("TRN2", target_bir_lowering=False, num_devices=num_devices)
    x_d = nc.dram_tensor("x", [B, C, N], f32, kind="ExternalInput").ap()
    wt_d = nc.dram_tensor("wt", [C, OW], bf16, kind="ExternalInput").ap()
    wot_d = nc.dram_tensor("wot", [4 * P, C], bf16, kind="ExternalInput").ap()
    xbf_d = nc.dram_tensor("xbf", [B, C, N], bf16, kind="ExternalInput").ap()
    cst_d = nc.dram_tensor("cst", [P, 16], f32, kind="ExternalInput").ap()
    cst4_d = nc.dram_tensor("cst4", [HEADS, 2 * P], f32, kind="ExternalInput").ap()
    out_d = nc.dram_tensor("out", [B, C, N], f32, kind="ExternalOutput").ap()

    with tile.TileContext(nc) as tc, ExitStack() as ctx:
        const = ctx.enter_context(tc.tile_pool(name="const", bufs=1))
        big = ctx.enter_context(tc.tile_pool(name="big", bufs=1))
        tmp = ctx.enter_context(tc.tile_pool(name="tmp", bufs=2))
        psA = ctx.enter_context(tc.tile_pool(name="psA", bufs=3, space="PSUM"))
        psB = ctx.enter_context(tc.tile_pool(name="psB", bufs=1, space="PSUM"))

        def mm(out, lhsT, rhs, start, stop, **kw):
            nc.tensor.matmul(out, lhsT, rhs, start=start, stop=stop, **kw)

        # ---------------- constants ----------------
        cst_f = tmp.tile([P, 16], f32, tag="cst_f", name="cst_f")
        nc.sync.dma_start(cst_f, cst_d[:])
        cst = const.tile([P, 16], bf16, tag="cst", name="cst")
        nc.vector.tensor_copy(out=cst[:], in_=cst_f[:])
        E_ind = [cst[:, 0:4], cst[:, 4:8]]
        rhs_x = cst[:, 8:10]
        rhs_q = cst[:, 10:12]
        negones = cst[:, 12:13]
        khalf = [cst[:, 13:14], cst[:, 15:16]]
        cst4_f = tmp.tile([HEADS, 2 * P], f32, tag="cst4_f", name="cst4_f")
        nc.sync.dma_start(cst4_f, cst4_d[:])
        cst4 = const.tile([HEADS, 2 * P], bf16, tag="cst4", name="cst4")
        nc.vector.tensor_copy(out=cst4[:], in_=cst4_f[:])
        E4 = [cst4[:, 0:128], cst4[:, 128:256]]

        ones2 = const.tile([P, D], bf16, tag="ones2", name="ones2")
        nc.vector.memset(ones2[:], 1.0)
        ones_row = const.tile([1, P], bf16, tag="ones_row", name="ones_row")
        nc.vector.memset(ones_row[:], 1.0)
        eps_col = const.tile([P, 1], f32, tag="eps_col", name="eps_col")
        nc.vector.memset(eps_col[:], LN_EPS)
        log8_col = const.tile([P, 1], f32, tag="log8_col", name="log8_col")
        nc.vector.memset(log8_col[:], LOG8)
        log8l_col = const.tile([P, 1], f32, tag="log8l_col", name="log8l_col")
        nc.vector.memset(log8l_col[:], LOG8L)

        # ---------------- PE warm-up ----------------
        # TRN2 PE DVFS ramps 0.65->1.2->2.4GHz only after ~3us of continuous
        # execution; idle resets it.  The first ~12us are DMA-bound with the
        # PE idle, so burn dummy matmuls there to enter the real work hot.
        warm = const.tile([P, 512], bf16, tag="warm", name="warm")
        nc.vector.memset(warm[:], 1.0)

        def _dummy_mm(n=1):
            for _ in range(n):
                wp = psA.tile([D, 512], f32, tag="A", name="wp")
                mm(wp[:], ones2[:, 0:D], warm[:], True, True)

        _dummy_mm(30)

        # ---------------- loads ----------------
        # xbf tiles first so stats can start ~2us in; x f32 loads deferred
        x_sb = [[big.tile([P, N], f32, tag=f"x{b}{cc}", name=f"x{b}{cc}")
                 for cc in range(2)] for b in range(B)]
        x_bf = [[big.tile([P, N], bf16, tag=f"xbf{b}{cc}", name=f"xbf{b}{cc}")
                 for cc in range(2)] for b in range(B)]
        xsq = [[big.tile([P, N], bf16, tag=f"xsq{b}{cc}", name=f"xsq{b}{cc}")
                for cc in range(2)] for b in range(B)]
        for b in range(B):
            for cc in range(2):
                nc.sync.dma_start(x_bf[b][cc], xbf_d[b, cc * P:(cc + 1) * P, :])
                nc.vector.tensor_mul(xsq[b][cc][:], x_bf[b][cc][:], x_bf[b][cc][:])

        wg_sb = big.tile([P, 2, OW], bf16, tag="wg", name="wg")
        nc.sync.dma_start(wg_sb, wt_d.rearrange("(cc p) o -> p cc o", p=P))
        wot_h = []
        for h in range(HEADS):
            wb = big.tile([P, C], bf16, tag=f"wot{h}", name=f"wot{h}")
            nc.sync.dma_start(wb, wot_d[h * P:(h + 1) * P, :])
            wot_h.append(wb)

        # ---------------- LN stats + centering (per batch) ----------------
        logrstd = [big.tile([P, 8], f32, tag=f"lrs{b}", name=f"lrs{b}") for b in range(B)]
        rstd_sb = [big.tile([P, 8], f32, tag=f"rstd{b}", name=f"rstd{b}") for b in range(B)]
        negmean_row = [big.tile([1, N], bf16, tag=f"nmr{b}", name=f"nmr{b}") for b in range(B)]

        def _stats_phase(b):
            st_ps = psA.tile([P, 8, 2], f32, tag="A", name="st_ps")
            for ic in range(8):
                sl = st_ps[:, ic]
                mm(sl, x_bf[b][0][:, ic * P:(ic + 1) * P], rhs_x, True, False)
                mm(sl, x_bf[b][1][:, ic * P:(ic + 1) * P], rhs_x, False, False)
                mm(sl, xsq[b][0][:, ic * P:(ic + 1) * P], rhs_q, False, False)
                mm(sl, xsq[b][1][:, ic * P:(ic + 1) * P], rhs_q, False, True)
            st_sb = tmp.tile([P, 8, 2], f32, tag="st_sb", name="st_sb")
            nc.vector.tensor_copy(out=st_sb[:], in_=st_ps[:])
            mean_v = st_sb[:, :, 0]
            msq_v = st_sb[:, :, 1]
            m2 = tmp.tile([P, 8], f32, tag="m2", name="m2")
            nc.vector.tensor_mul(m2[:], mean_v, mean_v)
            var = tmp.tile([P, 8], f32, tag="var", name="var")
            nc.vector.tensor_sub(var[:], msq_v, m2[:])
            lnv = tmp.tile([P, 8], f32, tag="lnv", name="lnv")
            nc.scalar.activation(lnv[:], var[:], AF.Ln, bias=eps_col[:])
            nc.vector.tensor_scalar_mul(logrstd[b][:], lnv[:], -0.5)
            nc.scalar.activation(rstd_sb[b][:], logrstd[b][:], AF.Exp)
            # negmean row + broadcast + center x in-place
            for ih in range(2):
                io = ih * 512
                nm_ps = psA.tile([1, 512], f32, tag="A", name="nm_ps")
                for cc in range(2):
                    mm(nm_ps[:], negones, x_bf[b][cc][:, io:io + 512],
                       start=(cc == 0), stop=(cc == 1))
                nc.vector.tensor_copy(out=negmean_row[b][:, io:io + 512], in_=nm_ps[:])
                nmbc_ps = psA.tile([P, 512], f32, tag="A", name="nmbc_ps")
                mm(nmbc_ps[:], ones_row[:],
                   negmean_row[b][:, io:io + 512], True, True)
                for cc in range(2):
                    nc.vector.tensor_add(x_bf[b][cc][:, io:io + 512],
                                         x_bf[b][cc][:, io:io + 512], nmbc_ps[:])

        # ---------------- QKV GEMM (q + padded k o-tiles) ----------------
        qkv_sb = [[big.tile([P, N], bf16, tag=f"qkv{b}{ot}", name=f"qkv{b}{ot}")
                   for ot in range(NQKV)] for b in range(B)]

        def _qkv_tile(b, ot, engine="v"):
            qk_ps = psA.tile([P, N], f32, tag="A", name="qk_ps")
            for ih in range(2):
                io = ih * 512
                mm(qk_ps[:, io:io + 512], wg_sb[:, 0, ot * P:(ot + 1) * P],
                   x_bf[b][0][:, io:io + 512], True, False)
                mm(qk_ps[:, io:io + 512], wg_sb[:, 1, ot * P:(ot + 1) * P],
                   x_bf[b][1][:, io:io + 512], False, True)
            if engine == "v":
                nc.vector.tensor_copy(out=qkv_sb[b][ot][:], in_=qk_ps[:])
            else:
                nc.scalar.copy(out=qkv_sb[b][ot][:], in_=qk_ps[:])

        # ---------------- v^T GEMM + augment (std col) ----------------
        # vaug[b]: [P(j), h, pair, sub, VW] e4m3; cols 0-63 v^T, 64 std,
        # 65-79 zero pad (DoubleRow LDWEIGHTS needs 16B-aligned subtile stride)
        VW = 80
        vaug = [big.tile([P, HEADS, 4, 2, VW], f8e4, tag=f"va{b}",
                         name=f"va{b}") for b in range(B)]
        for b in range(B):
            nc.gpsimd.memset(vaug[b][:, :, :, :, D:D + 1], 1.0)
            nc.gpsimd.memset(vaug[b][:, :, :, :, D + 1:VW], 0.0)

        def _vT_jc(b, jc):
            v_ps = psA.tile([P, 256], f32, tag="A", name="v_ps")
            for cc in range(2):
                mm(v_ps[:], x_bf[b][cc][:, jc * P:(jc + 1) * P],
                   wg_sb[:, cc, 768:1024], start=(cc == 0), stop=(cc == 1))
            p, s = jc // 2, jc % 2
            nc.vector.tensor_scalar_mul(
                vaug[b][:, :, p, s, 0:D],
                v_ps.rearrange("p (h d) -> p h d", h=HEADS),
                rstd_sb[b][:, jc:jc + 1])

        # ---------------- q/k norms ----------------
        b8_sb = [big.tile([P, 8, HEADS], f32, tag=f"b8{b}", name=f"b8{b}") for b in range(B)]
        b8s_sb = [big.tile([P, 8, HEADS], f32, tag=f"b8s{b}", name=f"b8s{b}") for b in range(B)]
        a_sb = [tmp.tile([HEADS, N], bf16, tag="a_sb", name=f"a_sb{b}") for b in range(B)]

        def _norm_phase(b):
            qsq = [tmp.tile([P, N], bf16, tag=f"qsq{cc}", name=f"qsq{cc}") for cc in range(2)]
            for cc in range(2):
                nc.vector.tensor_mul(qsq[cc][:], qkv_sb[b][cc][:], qkv_sb[b][cc][:])
            ksq = [tmp.tile([P, N], bf16, tag=f"ksq{h}", name=f"ksq{h}") for h in range(HEADS)]
            for h in range(HEADS):
                nc.vector.tensor_mul(ksq[h][:], qkv_sb[b][2 + h][:],
                                     qkv_sb[b][2 + h][:])
            a_ln = tmp.tile([HEADS, N], f32, tag="a_ln", name="a_ln")
            for ih in range(2):
                io = ih * 512
                s2q_ps = psA.tile([HEADS, 512], f32, tag="A", name="s2q_ps")
                for cc in range(2):
                    mm(s2q_ps[:], E_ind[cc], qsq[cc][:, io:io + 512],
                       start=(cc == 0), stop=(cc == 1))
                nc.scalar.activation(a_ln[:, io:io + 512], s2q_ps[:], AF.Ln)
            nc.scalar.activation(a_sb[b][:], a_ln[:], AF.Exp, scale=-0.5)
            bsq_ps = psA.tile([P, 8, HEADS], f32, tag="A", name="bsq_ps")
            for jc in range(8):
                for h in range(HEADS):
                    mm(bsq_ps[:, jc, h:h + 1], ksq[h][:, jc * P:(jc + 1) * P],
                       khalf[h % 2], True, True)
            b8ln = tmp.tile([P, 8, HEADS], f32, tag="b8ln", name="b8ln")
            nc.scalar.activation(b8ln[:], bsq_ps[:], AF.Ln)
            nc.scalar.activation(b8_sb[b][:], b8ln[:], AF.Exp, scale=-0.5,
                                 bias=log8_col[:])
            nc.scalar.activation(b8s_sb[b][:], b8ln[:], AF.Exp, scale=-0.5,
                                 bias=log8l_col[:])
            for cc in range(2):
                for ih in range(2):
                    io = ih * 512
                    abc_ps = psA.tile([P, 512], f32, tag="A", name="abc_ps")
                    mm(abc_ps[:], E4[cc], a_sb[b][:, io:io + 512], True, True)
                    nc.vector.tensor_mul(qkv_sb[b][cc][:, io:io + 512],
                                         qkv_sb[b][cc][:, io:io + 512], abc_ps[:])

        # ---------------- attention ----------------
        # et2 per head: [P(j), pair, sub, N(i)] u8 (== e5m2 bits)
        expp = ctx.enter_context(tc.tile_pool(name="expp", bufs=3))
        u65 = [[big.tile([D + 1, N], bf16, tag=f"u{b}{h}", name=f"u{b}{h}")
                for h in range(HEADS)] for b in range(B)]

        filler = []  # deferred closures: one popped after each attnV pair

        def _pop_filler(k=1):
            for _ in range(k):
                if filler:
                    filler.pop(0)()

        def _att_head(b, h):
            cc, hh = h // 2, h % 2
            dve_jcs = DVE_JCS[h]
            et2 = expp.tile([P, 4, 2, N], u8, tag="et2", name=f"et2_{b}{h}")
            et2_e5 = et2.bitcast(mybir.dt.float8e5)
            U_ps = psB.tile([VW, 2, 512], f32, tag="B", name="U_ps")

            def _attnV(p):
                for ih in range(2):
                    mm(U_ps[:, ih, :], vaug[b][:, h, p, :, :],
                       et2_e5[:, p, :, ih * 512:(ih + 1) * 512],
                       start=(p == 0), stop=(p == 3), perf_mode=DR)
                _pop_filler(1)

            for jc in range(8):
                p, s = jc // 2, jc % 2
                sim_ps = psA.tile([P, N], f32, tag="A", name="sim")
                for ih in range(2):
                    mm(sim_ps[:, ih * 512:(ih + 1) * 512],
                       qkv_sb[b][2 + h][:, jc * P:(jc + 1) * P],
                       qkv_sb[b][cc][:, ih * 512:(ih + 1) * 512],
                       True, True)
                if jc in dve_jcs:
                    nc.vector.tensor_scalar(
                        et2[:, p, s, :], sim_ps[:],
                        b8s_sb[b][:, jc, h:h + 1], 60.5,
                        op0=OP.mult, op1=OP.add)
                else:
                    nc.scalar.activation(et2_e5[:, p, s, :], sim_ps[:], AF.Exp,
                                         scale=b8_sb[b][:, jc, h:h + 1])
                # defer attnV of pair p until sims of pair p+1 are issued, so
                # the in-order PE queue never stalls waiting on the exp
                if s == 1 and p >= 1:
                    _attnV(p - 1)
            _attnV(3)
            nc.scalar.copy(
                out=u65[b][h].rearrange("p (i n) -> p i n", i=2),
                in_=U_ps[0:D + 1, :, :])

        def _epi_h(b, h):
            for ih in range(2):
                io = ih * 512
                sbc_ps = psA.tile([D, 512], f32, tag="A", name="sbc_ps")
                mm(sbc_ps[:], ones2[D:D + 1, :],
                   u65[b][h][D:D + 1, io:io + 512], True, True)
                rbc = tmp.tile([D, 512], f32, tag="rbc", name="rbc")
                nc.vector.reciprocal_approx_fast(out=rbc[:], in_=sbc_ps[:])
                nc.gpsimd.tensor_mul(u65[b][h][0:D, io:io + 512],
                                     u65[b][h][0:D, io:io + 512], rbc[:])

        def _proj(b):
            for co in range(2):
                out_f = tmp.tile([P, N], f32, tag="out_f", name="out_f")
                for ih in range(2):
                    io = ih * 512
                    out_ps = psA.tile([P, 512], f32, tag="A", name="out_ps")
                    for h in range(HEADS):
                        mm(out_ps[:], wot_h[h][0:D, co * P:(co + 1) * P],
                           u65[b][h][0:D, io:io + 512],
                           start=(h == 0), stop=(h == 3))
                    nc.vector.tensor_add(out_f[:, io:io + 512],
                                         out_ps[:], x_sb[b][co][:, io:io + 512])
                nc.sync.dma_start(out_d[b, co * P:(co + 1) * P, :], out_f[:])

        # ---------------- schedule ----------------
        _stats_phase(0)
        for ot in (2, 3, 4, 5, 0, 1):
            _qkv_tile(0, ot)
        for jc in range(8):
            _vT_jc(0, jc)
        _norm_phase(0)
        for b in range(B):
            for cc in range(2):
                nc.sync.dma_start(x_sb[b][cc], x_d[b, cc * P:(cc + 1) * P, :])

        # batch-1 prep runs as filler inside batch-0's attention heads
        filler.append(lambda: _stats_phase(1))
        for ot in (2, 3):
            filler.append(lambda ot=ot: _qkv_tile(1, ot, engine="s"))
        for ot in (4, 5):
            filler.append(lambda ot=ot: _qkv_tile(1, ot))
        for jc in range(0, 8, 2):
            filler.append(lambda jc=jc: (_vT_jc(1, jc), _vT_jc(1, jc + 1)))
        for ot in (0, 1):
            filler.append(lambda ot=ot: _qkv_tile(1, ot))
        filler.append(lambda: _norm_phase(1))

        for h in range(HEADS):
            _att_head(0, h)
        _pop_filler(16)
        _att_head(1, 0)
        _epi_h(0, 0)
        _att_head(1, 1)
        _epi_h(0, 1)
        _epi_h(1, 0)
        _att_head(1, 2)
        _epi_h(0, 2)
        _epi_h(1, 1)
        _att_head(1, 3)
        _epi_h(0, 3)
        _proj(0)
        _epi_h(1, 2)
        _epi_h(1, 3)
        _proj(1)

    nc.compile()
    return nc


_NC = None
last_exec_time_ns = None


def _get_nc():
    global _NC
    if _NC is None:
        _NC = build_nc()
    return _NC


def _run(in_maps, trace=False):
    global last_exec_time_ns
    from concourse.bass_utils import run_bass_kernel_spmd
    nc = _get_nc()
    if trace:
        _install_ntff_hook()
    try:
        res = run_bass_kernel_spmd(nc, in_maps, core_ids=list(range(NCORES)),
                                   trace=trace)
    except Exception:
        if not trace:
            raise
        res = run_bass_kernel_spmd(nc, in_maps, core_ids=list(range(NCORES)),
                                   trace=False)
    last_exec_time_ns = res.exec_time_ns
    return res


def make_in_maps(x, g, w_qkv, w_out, ncores=NCORES):
    import ml_dtypes as _md
    x = np.ascontiguousarray(np.asarray(x, dtype=np.float32))
    g = np.asarray(g, dtype=np.float32).reshape(C)
    wt_pad, wot_pad = _host_weights(w_qkv, w_out, g)
    b_full = x.shape[0]
    xr = x.reshape(b_full, C, N)
    cst, cst4 = _host_consts()
    in_maps = []
    for i in range(ncores):
        in_maps.append({
            "x": np.ascontiguousarray(xr[i * B:(i + 1) * B]),
            "xbf": np.ascontiguousarray(xr[i * B:(i + 1) * B]).astype(_md.bfloat16),
            "wt": wt_pad,
            "wot": wot_pad,
            "cst": cst,
            "cst4": cst4,
        })
    return in_maps


def kernel(x, g, w_qkv, w_out, _trace=False):
    x = np.ascontiguousarray(np.asarray(x, dtype=np.float32))
    b_full, c, H, W = x.shape
    assert (b_full, c, H * W) == (NCORES * B, C, N)
    in_maps = make_in_maps(x, g, w_qkv, w_out)
    res = _run(in_maps, trace=_trace)
    out = np.concatenate([res.results[i]["out"] for i in range(NCORES)], axis=0)
    return out.reshape(b_full, C, H, W).astype(np.float32)
